# revision 12
# baseline (speedup 1.0000x reference)
"""ClassBalancedSupConLoss on 8 TRN2 NeuronCores (Bass/Tile) — v3.

The kernel is exp-throughput-bound. Two engines share the exp stream:
  - ACT: LUT Exp at 1 col/cycle @1.2GHz, accum_out per segment;
  - DVE: Schraudolph bit-trick exp
        exp(inv_t*(s-1)) ~= f32_from_bits(int32(s*A + B))
    as pass1 (PSUM->SBUF i32 convert) + pass2 (bitcast f32, accum_out),
    both at 1 col/cycle @0.96GHz.
With a WARM PE (HAM K=8/8), chunk fills (~0.9us) hide entirely under
the 1.6-2.3us consumer calls, so the stream runs at the combined
consumer rate (~1.9x the ACT-only baseline). Keeping the PE warm is
load-bearing: a dedicated 512-col junk PSUM bank takes filler matmuls
after every chunk so the HAM activity window never sees an idle PE.

PSUM layout: ping [128,2048] (banks 0-3) | pong [128,1536] (banks 4-6)
| junk [128,512] (bank 7). Chunks alternate ping/pong:
  [bb 2048 | 1536 | 2048 | 1536 | ... | 1536 | 512]  (11 chunks/tile)
Class-boundary-straddling chunks go to the DVE when possible (its
accum-range splits are nearly free).

The rest (sorted batch/bank, s_ii self-term cancellation via prelude
diag matmul, positives as matmuls against per-class sum vectors, host
final log + masked mean) matches the earlier scheme.
"""

import os
import numpy as np

import concourse.bass as bass  # noqa: F401
from concourse import bacc
import concourse.mybir as mybir
import concourse.tile as tile
from concourse.bass_utils import run_bass_kernel_spmd

B, D, M, C = 2048, 128, 16384, 3
NCORES = 8
APC = B // NCORES          # anchors per core = 256
NT = APC // 128            # anchor tiles per core = 2
CH = 512
G, P = 2048, 1536          # ping / pong chunk widths
BASE_TEMP = 0.07

F32 = mybir.dt.float32
I32 = mybir.dt.int32
BF16 = mybir.dt.bfloat16
AF = mybir.ActivationFunctionType
ALU = mybir.AluOpType
AX = mybir.AxisListType

MM_MODE = os.environ.get("SUPCON_MM_MODE", "bf16")
N_DVE = int(os.environ.get("SUPCON_DVE", "3"))     # DVE chunks per tile
N_JUNK = int(os.environ.get("SUPCON_JUNK", "2"))   # junk MMs per chunk

LAST_EXEC_TIME_NS = None

K_SCH = float(2.0 ** 23 / np.log(2.0))
MAGIC = 127.0 * 2 ** 23


def _schraudolph_C():
    """C with zero mean multiplicative error: ratio(f) = (1+f)/2^(f+c)."""
    f = np.linspace(0.0, 1.0, 200001)[:-1]
    mean_i = np.mean((1.0 + f) / np.exp2(f))
    return float(np.log2(mean_i) * 2.0 ** 23)


C_SCH = _schraudolph_C()


def _install_trace_shim():
    """Register the NTFF profile hook that this image's antenv lacks."""
    import sys
    import types
    import ctypes
    import contextlib

    try:
        from antenv.axon_hooks import get_axon_ntff_profile_hook  # noqa: F401
        return True
    except ImportError:
        pass

    so_path = "/opt/axon/libaxon_pjrt.so"
    if not os.path.exists(so_path):
        return False
    lib = ctypes.CDLL(so_path)
    if not hasattr(lib, "axon_start_nrt_profile"):
        return False
    lib.axon_start_nrt_profile.argtypes = [
        ctypes.POINTER(ctypes.c_int64),
        ctypes.c_size_t,
    ]
    lib.axon_start_nrt_profile.restype = ctypes.c_int64
    lib.axon_stop_nrt_profile.argtypes = [ctypes.c_char_p]
    lib.axon_stop_nrt_profile.restype = ctypes.c_int64

    @contextlib.contextmanager
    def _hook(output_dir, device_ids):
        import jax

        jax.devices()
        if device_ids:
            ids = (ctypes.c_int64 * len(device_ids))(*device_ids)
            rc = lib.axon_start_nrt_profile(ids, len(device_ids))
        else:
            rc = lib.axon_start_nrt_profile(None, 0)
        if rc != 0:
            raise RuntimeError(f"axon_start_nrt_profile rc={rc}")
        try:
            yield
        finally:
            n = lib.axon_stop_nrt_profile(str(output_dir).encode())
            print(f"profile: {n} file(s) written to {output_dir}", file=sys.stderr)

    _state = {"hook": _hook}
    mod = types.ModuleType("antenv.axon_hooks")
    mod.get_axon_ntff_profile_hook = lambda: _state["hook"]
    mod.set_axon_ntff_profile_hook = lambda h: _state.update(hook=h)
    sys.modules["antenv.axon_hooks"] = mod
    import antenv

    antenv.axon_hooks = mod

    import concourse.bass_utils as bu

    bu.upload_artifacts = lambda tmpdir: tmpdir
    return True


def _chunk_list():
    """[(buf, bank_start, size)]; bank_start=-1 is the bb chunk."""
    chunks = [("G", -1, G)]
    pos = 0
    sizes = [P, G, P, G, P, G, P, G, P, 512]
    for i, sz in enumerate(sizes):
        chunks.append(("P" if i % 2 == 0 else "G", pos, sz))
        pos += sz
    assert pos == M
    return chunks


def _plan(mk_b1, mk_b2, n_dve):
    """Per-chunk accumulate-segments, DVE assignment, global slot list."""
    chunks = _chunk_list()
    segs = []           # per chunk: [(a, b)] local col ranges
    for (buf, s, sz) in chunks:
        if s < 0:
            segs.append([(0, sz)])
            continue
        cuts = {s, s + sz}
        for bnd in (mk_b1, mk_b2):
            if s < bnd < s + sz:
                cuts.add(bnd)
        cuts = sorted(cuts)
        segs.append([(cuts[i] - s, cuts[i + 1] - s) for i in range(len(cuts) - 1)])

    # DVE chunks: boundary chunks first, then big chunks, never adjacent,
    # never the bb chunk (0) or the 512 tail.
    cand = [i for i in range(1, len(chunks)) if len(segs[i]) > 1]
    cand += [i for i in (2, 6, 4, 8, 1, 5, 3, 7, 9) if i not in cand]
    dve = []
    for i in cand:
        if len(dve) >= n_dve:
            break
        if i == 0 or chunks[i][2] < 1024:
            continue
        if any(abs(i - j) <= 1 for j in dve):
            continue
        dve.append(i)

    slots = []
    slot_idx = {}
    for ci, (buf, s, sz) in enumerate(chunks):
        for (a, b) in segs[ci]:
            if s < 0:
                cls = -1
            else:
                ge = s + b
                cls = 0 if ge <= mk_b1 else (1 if ge <= mk_b2 else 2)
            slot_idx[(ci, a)] = len(slots)
            slots.append((ci, a, b, cls))
    return chunks, segs, set(dve), slots, slot_idx


def _build(mk_b1, mk_b2, n_dve, use_dve):
    import ml_dtypes  # noqa: F401

    in_dt = BF16 if MM_MODE == "bf16" else F32

    chunks, segs, dve_set, slots, slot_idx = _plan(mk_b1, mk_b2, n_dve)
    if not use_dve:
        dve_set = set()
    NSLOT = len(slots)

    nc = bacc.Bacc()
    embT_d = nc.declare_dram_parameter("embT", [D, B], in_dt, isOutput=False)
    anchT_d = nc.declare_dram_parameter("anchT", [D, APC + C], in_dt, isOutput=False)
    bankT_d = nc.declare_dram_parameter("bankT", [D, M], in_dt, isOutput=False)
    NV = NT * (6 + C + NSLOT) + 128
    vecs_d = nc.declare_dram_parameter("vecs", [128, NV], F32, isOutput=False)
    oout_d = nc.declare_dram_parameter("oout", [128, 2 * NT], F32, isOutput=True)

    with tile.TileContext(nc) as tc:
        with (
            tc.tile_pool(name="big", bufs=1) as bigp,
            tc.tile_pool(name="sm", bufs=1) as smp,
            tc.tile_pool(name="ping", bufs=1, space="PSUM") as pingp,
            tc.tile_pool(name="pong", bufs=1, space="PSUM") as pongp,
            tc.tile_pool(name="junk", bufs=1, space="PSUM") as junkp,
        ):
            anch_t = bigp.tile([D, APC + C], in_dt, tag="anchT")
            vecs_t = smp.tile([128, NV], F32, tag="vecs")
            junkw_t = bigp.tile([128, 128], in_dt, tag="junkw")
            junkx_t = bigp.tile([128, CH], in_dt, tag="junkx")
            o = [0]

            def vslice(w):
                a = o[0]
                o[0] += w
                return vecs_t[:, a:a + w]

            invt_t = vslice(NT)
            ninvt_t = vslice(NT)
            invpc_t = vslice(NT)
            coefv_t = vslice(NT)
            asch_t = vslice(NT)
            bsch_t = vslice(NT)
            oneh_t = vslice(NT * C)
            incl_t = vslice(NT * NSLOT)
            eye_t = vslice(128)

            emb_t = bigp.tile([D, B], in_dt, tag="embT")
            bank_t = bigp.tile([D, M], in_dt, tag="bankT")

            # DMA triggers: sync + scalar HWDGE; the scalar ones all fire
            # before the exp stream starts.
            Q = B // 4
            H4 = M // 4
            nc.sync.dma_start(out=vecs_t[:], in_=vecs_d[:])
            nc.sync.dma_start(out=anch_t[:], in_=anchT_d[:])
            nc.sync.dma_start(out=emb_t[:, 0:Q], in_=embT_d[:, 0:Q])
            nc.sync.dma_start(out=emb_t[:, Q:2 * Q], in_=embT_d[:, Q:2 * Q])
            nc.scalar.dma_start(out=emb_t[:, 2 * Q:3 * Q], in_=embT_d[:, 2 * Q:3 * Q])
            nc.scalar.dma_start(out=emb_t[:, 3 * Q:B], in_=embT_d[:, 3 * Q:B])
            nc.sync.dma_start(out=bank_t[:, 0:H4], in_=bankT_d[:, 0:H4])
            nc.scalar.dma_start(out=bank_t[:, H4:2 * H4], in_=bankT_d[:, H4:2 * H4])
            nc.sync.dma_start(out=bank_t[:, 2 * H4:3 * H4], in_=bankT_d[:, 2 * H4:3 * H4])
            nc.scalar.dma_start(out=bank_t[:, 3 * H4:M], in_=bankT_d[:, 3 * H4:M])

            oout_t = smp.tile([128, 2 * NT], F32, tag="oout")
            scrA = smp.tile([128, G], BF16, tag="scrA")
            scrI = smp.tile([128, G], I32, tag="scrI")
            scrO2 = smp.tile([128, G], BF16, tag="scrO2")
            sdiag = [smp.tile([128, 1], F32, tag=f"sdiag{t}", name=f"sdiag{t}") for t in range(NT)]
            selfe = [smp.tile([128, 1], F32, tag=f"selfe{t}", name=f"selfe{t}") for t in range(NT)]
            eyemul = smp.tile([128, 128], F32, tag="eyemul")
            warm = smp.tile([128, 1], F32, tag="warm")
            raw3 = [smp.tile([128, C], F32, tag=f"raw3{t}", name=f"raw3{t}") for t in range(NT)]
            esum = [smp.tile([128, NSLOT], F32, tag=f"esum{t}", name=f"esum{t}") for t in range(NT)]
            scrNK = [smp.tile([128, NSLOT], F32, tag=f"scrNK{t}", name=f"scrNK{t}") for t in range(NT)]
            scrC = [smp.tile([128, C], F32, tag=f"scrC{t}", name=f"scrC{t}") for t in range(NT)]

            # exp table load ASAP (no DMA dependency)
            nc.vector.memset(junkw_t[:], 0.0)
            nc.vector.memset(junkx_t[:], 0.0)
            nc.scalar.activation(warm[:], junkw_t[:, 0:1], AF.Exp)

            def anch(t):
                return anch_t[:, t * 128:(t + 1) * 128]

            junk_ps = junkp.tile([128, CH], F32, tag="junk", name="junk_ps")

            def emit_junk(n, dep=None):
                # dep: an SBUF AP written by the chunk's consumer. Using it
                # as the weights pins the junk MM *after* that consumer in
                # the dependency graph, so the scheduler can't hoist the
                # filler ahead of real work — it runs exactly in the PE
                # idle window while the next chunk's fill is blocked.
                w = junkw_t[:] if dep is None else dep
                for _ in range(n):
                    nc.tensor.matmul(junk_ps[:], w, junkx_t[:],
                                     start=True, stop=True)

            # PE warmup while DMAs land
            emit_junk(8)

            # prelude in the pong buffer: diag blocks + per-class raw sums
            pre_ps = pongp.tile([128, P], F32, tag="pong", name="pre_ps")
            for t in range(NT):
                nc.tensor.matmul(
                    pre_ps[:, t * 128:(t + 1) * 128], anch(t), anch(t),
                    start=True, stop=True,
                )
            for t in range(NT):
                nc.tensor.matmul(
                    pre_ps[:, 256 + t * C:256 + (t + 1) * C], anch(t),
                    anch_t[:, APC:APC + C], start=True, stop=True,
                )
            for t in range(NT):
                nc.vector.tensor_mul(eyemul[:], pre_ps[:, t * 128:(t + 1) * 128], eye_t[:])
                nc.vector.reduce_sum(sdiag[t][:], eyemul[:], axis=AX.X)
                nc.vector.tensor_copy(out=raw3[t][:], in_=pre_ps[:, 256 + t * C:256 + (t + 1) * C])
                nc.scalar.activation(
                    selfe[t][:], sdiag[t][:], AF.Exp,
                    bias=ninvt_t[:, t:t + 1], scale=invt_t[:, t:t + 1],
                )

            def epi_early(t):
                """olin = coefv*invt*(1 - pos); runs during the stream."""
                own_r = smp.tile([128, 1], F32, tag=f"ownr{t}", name=f"ownr{t}")
                pos = smp.tile([128, 1], F32, tag=f"pos{t}", name=f"pos{t}")
                w1 = smp.tile([128, 1], F32, tag=f"w1{t}", name=f"w1{t}")
                nc.vector.tensor_mul(scrC[t][:], raw3[t][:], oneh_t[:, t * C:(t + 1) * C])
                nc.vector.reduce_sum(own_r[:], scrC[t][:], axis=AX.X)
                nc.vector.scalar_tensor_tensor(
                    out=pos[:], in0=own_r[:], scalar=sdiag[t][:], in1=invpc_t[:, t:t + 1],
                    op0=ALU.subtract, op1=ALU.mult,
                )
                nc.vector.scalar_tensor_tensor(
                    out=w1[:], in0=pos[:], scalar=-1.0, in1=invt_t[:, t:t + 1],
                    op0=ALU.mult, op1=ALU.mult,
                )
                nc.vector.scalar_tensor_tensor(
                    out=oout_t[:, NT + t:NT + t + 1], in0=w1[:], scalar=invt_t[:, t:t + 1],
                    in1=coefv_t[:, t:t + 1], op0=ALU.add, op1=ALU.mult,
                )

            def epilogue(t):
                """den = sum_k esum_k*incl_k - selfe."""
                nc.vector.tensor_mul(scrNK[t][:], esum[t][:], incl_t[:, t * NSLOT:(t + 1) * NSLOT])
                nc.vector.reduce_sum(oout_t[:, t:t + 1], scrNK[t][:], axis=AX.X)
                nc.vector.tensor_sub(oout_t[:, t:t + 1], oout_t[:, t:t + 1], selfe[t][:])

            def emit_chunk(t, ci, junk=True):
                buf, s, sz = chunks[ci]
                pool, tg, bw = (pingp, "ping", G) if buf == "G" else (pongp, "pong", P)
                ps = pool.tile([128, bw], F32, tag=tg, name=f"ps_t{t}_c{ci}")
                off = 0 if s < 0 else s
                src = emb_t if s < 0 else bank_t
                nmm = (sz + CH - 1) // CH
                for q in range(nmm):
                    a, b = q * CH, min((q + 1) * CH, sz)
                    nc.tensor.matmul(
                        ps[:, a:b], anch(t), src[:, off + a:off + b],
                        start=True, stop=True,
                    )
                if ci in dve_set:
                    nc.vector.tensor_scalar(
                        out=scrI[:, 0:sz], in0=ps[:, 0:sz],
                        scalar1=asch_t[:, t:t + 1], scalar2=bsch_t[:, t:t + 1],
                        op0=ALU.mult, op1=ALU.add,
                    )
                    for (a, b) in segs[ci]:
                        k = slot_idx[(ci, a)]
                        nc.vector.tensor_scalar(
                            out=scrO2[:, a:b], in0=scrI[:, a:b].bitcast(F32),
                            scalar1=1.0, scalar2=0.0,
                            op0=ALU.mult, op1=ALU.add,
                            accum_out=esum[t][:, k:k + 1],
                        )
                    if junk:
                        dep = (scrI[:, 0:64].bitcast(BF16) if in_dt == BF16
                               else scrI[:, 0:128].bitcast(F32))
                        emit_junk(N_JUNK, dep=dep)
                else:
                    for (a, b) in segs[ci]:
                        k = slot_idx[(ci, a)]
                        nc.scalar.activation(
                            scrA[:, a:b], ps[:, a:b], AF.Exp,
                            bias=ninvt_t[:, t:t + 1], scale=invt_t[:, t:t + 1],
                            accum_out=esum[t][:, k:k + 1],
                        )
                    if junk:
                        dep = (scrA[:, 0:128] if in_dt == BF16
                               else scrA[:, 0:256].bitcast(F32))
                        emit_junk(N_JUNK, dep=dep)

            NCH = len(chunks)
            emit_chunk(0, 0, junk=False)
            emit_chunk(0, 1, junk=False)
            for t in range(NT):
                epi_early(t)
            for ci in range(2, NCH):
                emit_chunk(0, ci)
            emit_chunk(1, 0)
            emit_chunk(1, 1)
            epilogue(0)
            for ci in range(2, NCH):
                emit_chunk(1, ci)
            epilogue(1)

            nc.sync.dma_start(out=oout_d[:], in_=oout_t[:])

    nc.compile()
    return nc, slots, NSLOT


def _per_core_cols(vec, core):
    """[B] host vector -> [128, NT] tile for one core (col t, partition p)."""
    sl = vec[core * APC:(core + 1) * APC]
    return np.ascontiguousarray(sl.reshape(NT, 128).T).astype(np.float32)


def kernel(embeddings, labels, bank_embs, bank_labels, class_temps):
    global LAST_EXEC_TIME_NS
    import ml_dtypes

    emb = np.asarray(embeddings, dtype=np.float32)
    bank = np.asarray(bank_embs, dtype=np.float32)
    lab = np.asarray(labels).astype(np.int64).ravel()
    blab = np.asarray(bank_labels).astype(np.int64).ravel()
    ct = np.asarray(class_temps, dtype=np.float32).ravel()

    bord = np.argsort(lab, kind="stable")
    slab = lab[bord]
    mord = np.argsort(blab, kind="stable")
    cnt = np.bincount(lab, minlength=C)
    mcnt = np.bincount(blab, minlength=C)
    mk_b1, mk_b2 = int(mcnt[0]), int(mcnt[0] + mcnt[1])

    embT = np.ascontiguousarray(emb[bord].T)      # [D, B]
    bankT = np.ascontiguousarray(bank[mord].T)    # [D, M]
    if MM_MODE == "bf16":
        embT = embT.astype(ml_dtypes.bfloat16)
        bankT = bankT.astype(ml_dtypes.bfloat16)

    temps = ct[slab]
    inv_t = (1.0 / temps).astype(np.float32)
    use_dve = N_DVE > 0 and float(inv_t.max()) <= 40.0
    pos_cnt = cnt[slab] - 1
    invpc = (1.0 / np.maximum(pos_cnt, 1)).astype(np.float32)
    validf = (pos_cnt > 0).astype(np.float32)
    coefv = (BASE_TEMP / temps).astype(np.float32) * validf
    oneh = np.eye(C, dtype=np.float32)[slab]      # [B, 3]
    n_valid = int((pos_cnt > 0).sum())

    nc, slots, NSLOT = _build(mk_b1, mk_b2, N_DVE, use_dve)

    slot_cls = np.array([cls for (_, _, _, cls) in slots])
    incl_full = ((slot_cls[None, :] < 0) | (slot_cls[None, :] != slab[:, None])).astype(np.float32)
    eye128 = np.eye(128, dtype=np.float32)

    asch = (K_SCH * inv_t).astype(np.float32)
    bsch = (MAGIC - C_SCH - K_SCH * inv_t.astype(np.float64)).astype(np.float32)

    gT = np.stack([emb[bord][slab == c].sum(axis=0) for c in range(C)], axis=1)
    gT = np.ascontiguousarray(gT).astype(embT.dtype)

    in_maps = []
    for core in range(NCORES):
        asl = slice(core * APC, (core + 1) * APC)
        oh = oneh[asl].reshape(NT, 128, C).transpose(1, 0, 2).reshape(128, NT * C)
        ic = incl_full[asl].reshape(NT, 128, NSLOT).transpose(1, 0, 2).reshape(128, NT * NSLOT)
        vecs = np.concatenate([
            _per_core_cols(inv_t, core),
            _per_core_cols(-inv_t, core),
            _per_core_cols(invpc, core),
            _per_core_cols(coefv, core),
            _per_core_cols(asch, core),
            _per_core_cols(bsch, core),
            oh.astype(np.float32),
            ic.astype(np.float32),
            eye128,
        ], axis=1)
        in_maps.append({
            "embT": embT,
            "anchT": np.ascontiguousarray(np.concatenate([embT[:, asl], gT], axis=1)),
            "bankT": bankT,
            "vecs": np.ascontiguousarray(vecs),
        })

    trace = os.environ.get("SUPCON_TRACE", "0") == "1"
    if trace:
        trace = _install_trace_shim()
    res = run_bass_kernel_spmd(nc, in_maps, core_ids=list(range(NCORES)), trace=trace)
    LAST_EXEC_TIME_NS = res.exec_time_ns

    loss_sum = np.float64(0.0)
    for core in range(NCORES):
        oo = np.asarray(res.results[core]["oout"], dtype=np.float64)    # [128, 2*NT]
        den, lin = oo[:, :NT], oo[:, NT:]
        cf = _per_core_cols(coefv, core).astype(np.float64)
        loss_sum += (cf * np.log(den) + lin).sum()
    return np.float32(loss_sum / max(n_valid, 1))


# revision 16
# speedup vs baseline: 1.2695x; 1.2695x over previous
"""ClassBalancedSupConLoss on 8 TRN2 NeuronCores (Bass/Tile) — v3.

The kernel is exp-throughput-bound. Two engines share the exp stream:
  - ACT: LUT Exp at 1 col/cycle @1.2GHz, accum_out per segment;
  - DVE: Schraudolph bit-trick exp
        exp(inv_t*(s-1)) ~= f32_from_bits(int32(s*A + B))
    as pass1 (PSUM->SBUF i32 convert) + pass2 (bitcast f32, accum_out),
    both at 1 col/cycle @0.96GHz.
With a WARM PE (HAM K=8/8), chunk fills (~0.9us) hide entirely under
the 1.6-2.3us consumer calls, so the stream runs at the combined
consumer rate (~1.9x the ACT-only baseline). Keeping the PE warm is
load-bearing: a dedicated 512-col junk PSUM bank takes filler matmuls
after every chunk so the HAM activity window never sees an idle PE.

PSUM layout: ping [128,2048] (banks 0-3) | pong [128,1536] (banks 4-6)
| junk [128,512] (bank 7). Chunks alternate ping/pong:
  [bb 2048 | 1536 | 2048 | 1536 | ... | 1536 | 512]  (11 chunks/tile)
Class-boundary-straddling chunks go to the DVE when possible (its
accum-range splits are nearly free).

The rest (sorted batch/bank, s_ii self-term cancellation via prelude
diag matmul, positives as matmuls against per-class sum vectors, host
final log + masked mean) matches the earlier scheme.
"""

import os
import numpy as np

import concourse.bass as bass  # noqa: F401
from concourse import bacc
import concourse.mybir as mybir
import concourse.tile as tile
from concourse.bass_utils import run_bass_kernel_spmd

B, D, M, C = 2048, 128, 16384, 3
NCORES = 8
APC = B // NCORES          # anchors per core = 256
NT = APC // 128            # anchor tiles per core = 2
CH = 512
G, P = 2048, 1536          # ping / pong chunk widths
BASE_TEMP = 0.07

F32 = mybir.dt.float32
I32 = mybir.dt.int32
BF16 = mybir.dt.bfloat16
AF = mybir.ActivationFunctionType
ALU = mybir.AluOpType
AX = mybir.AxisListType

MM_MODE = os.environ.get("SUPCON_MM_MODE", "bf16")
N_DVE = int(os.environ.get("SUPCON_DVE", "3"))     # DVE chunks per tile
N_JUNK = int(os.environ.get("SUPCON_JUNK", "2"))   # junk MMs per chunk

LAST_EXEC_TIME_NS = None

K_SCH = float(2.0 ** 23 / np.log(2.0))
MAGIC = 127.0 * 2 ** 23


def _schraudolph_C():
    """C with zero mean multiplicative error: ratio(f) = (1+f)/2^(f+c)."""
    f = np.linspace(0.0, 1.0, 200001)[:-1]
    mean_i = np.mean((1.0 + f) / np.exp2(f))
    return float(np.log2(mean_i) * 2.0 ** 23)


C_SCH = _schraudolph_C()


def _install_trace_shim():
    """Register the NTFF profile hook that this image's antenv lacks."""
    import sys
    import types
    import ctypes
    import contextlib

    try:
        from antenv.axon_hooks import get_axon_ntff_profile_hook  # noqa: F401
        return True
    except ImportError:
        pass

    so_path = "/opt/axon/libaxon_pjrt.so"
    if not os.path.exists(so_path):
        return False
    lib = ctypes.CDLL(so_path)
    if not hasattr(lib, "axon_start_nrt_profile"):
        return False
    lib.axon_start_nrt_profile.argtypes = [
        ctypes.POINTER(ctypes.c_int64),
        ctypes.c_size_t,
    ]
    lib.axon_start_nrt_profile.restype = ctypes.c_int64
    lib.axon_stop_nrt_profile.argtypes = [ctypes.c_char_p]
    lib.axon_stop_nrt_profile.restype = ctypes.c_int64

    @contextlib.contextmanager
    def _hook(output_dir, device_ids):
        import jax

        jax.devices()
        if device_ids:
            ids = (ctypes.c_int64 * len(device_ids))(*device_ids)
            rc = lib.axon_start_nrt_profile(ids, len(device_ids))
        else:
            rc = lib.axon_start_nrt_profile(None, 0)
        if rc != 0:
            raise RuntimeError(f"axon_start_nrt_profile rc={rc}")
        try:
            yield
        finally:
            n = lib.axon_stop_nrt_profile(str(output_dir).encode())
            print(f"profile: {n} file(s) written to {output_dir}", file=sys.stderr)

    _state = {"hook": _hook}
    mod = types.ModuleType("antenv.axon_hooks")
    mod.get_axon_ntff_profile_hook = lambda: _state["hook"]
    mod.set_axon_ntff_profile_hook = lambda h: _state.update(hook=h)
    sys.modules["antenv.axon_hooks"] = mod
    import antenv

    antenv.axon_hooks = mod

    import concourse.bass_utils as bu

    bu.upload_artifacts = lambda tmpdir: tmpdir
    return True


def _chunk_list():
    """[(buf, bank_start, size)]; bank_start=-1 is the bb chunk."""
    chunks = [("G", -1, G)]
    pos = 0
    sizes = [P, G, P, G, P, G, P, G, P, 512]
    for i, sz in enumerate(sizes):
        chunks.append(("P" if i % 2 == 0 else "G", pos, sz))
        pos += sz
    assert pos == M
    return chunks


def _plan(mk_b1, mk_b2, n_dve):
    """Per-chunk accumulate-segments, DVE assignment, global slot list."""
    chunks = _chunk_list()
    segs = []           # per chunk: [(a, b)] local col ranges
    for (buf, s, sz) in chunks:
        if s < 0:
            segs.append([(0, sz)])
            continue
        cuts = {s, s + sz}
        for bnd in (mk_b1, mk_b2):
            if s < bnd < s + sz:
                cuts.add(bnd)
        cuts = sorted(cuts)
        segs.append([(cuts[i] - s, cuts[i + 1] - s) for i in range(len(cuts) - 1)])

    # DVE chunks: boundary chunks first, then big chunks, never adjacent,
    # never the bb chunk (0) or the 512 tail.
    cand = [i for i in range(1, len(chunks)) if len(segs[i]) > 1]
    cand += [i for i in (2, 6, 4, 8, 1, 5, 3, 7, 9) if i not in cand]
    dve = []
    for i in cand:
        if len(dve) >= n_dve:
            break
        if i == 0 or chunks[i][2] < 1024:
            continue
        if any(abs(i - j) <= 1 for j in dve):
            continue
        dve.append(i)

    slots = []
    slot_idx = {}
    for ci, (buf, s, sz) in enumerate(chunks):
        for (a, b) in segs[ci]:
            if s < 0:
                cls = -1
            else:
                ge = s + b
                cls = 0 if ge <= mk_b1 else (1 if ge <= mk_b2 else 2)
            slot_idx[(ci, a)] = len(slots)
            slots.append((ci, a, b, cls))
    return chunks, segs, set(dve), slots, slot_idx


def _build(mk_b1, mk_b2, n_dve, use_dve):
    import ml_dtypes  # noqa: F401

    in_dt = BF16 if MM_MODE == "bf16" else F32

    chunks, segs, dve_set, slots, slot_idx = _plan(mk_b1, mk_b2, n_dve)
    if not use_dve:
        dve_set = set()
    NSLOT = len(slots)

    nc = bacc.Bacc()
    embT_d = nc.declare_dram_parameter("embT", [D, B], in_dt, isOutput=False)
    anchT_d = nc.declare_dram_parameter("anchT", [D, APC + C], in_dt, isOutput=False)
    bankT_d = nc.declare_dram_parameter("bankT", [D, M], in_dt, isOutput=False)
    NV = NT * (6 + C + NSLOT) + 128
    vecs_d = nc.declare_dram_parameter("vecs", [128, NV], F32, isOutput=False)
    oout_d = nc.declare_dram_parameter("oout", [128, 2 * NT], F32, isOutput=True)

    with tile.TileContext(nc) as tc:
        with (
            tc.tile_pool(name="big", bufs=1) as bigp,
            tc.tile_pool(name="sm", bufs=1) as smp,
            tc.tile_pool(name="ping", bufs=1, space="PSUM") as pingp,
            tc.tile_pool(name="pong", bufs=1, space="PSUM") as pongp,
            tc.tile_pool(name="junk", bufs=1, space="PSUM") as junkp,
        ):
            anch_t = bigp.tile([D, APC + C], in_dt, tag="anchT")
            vecs_t = smp.tile([128, NV], F32, tag="vecs")
            junkw_t = bigp.tile([128, 128], in_dt, tag="junkw")
            junkx_t = bigp.tile([128, CH], in_dt, tag="junkx")
            o = [0]

            def vslice(w):
                a = o[0]
                o[0] += w
                return vecs_t[:, a:a + w]

            invt_t = vslice(NT)
            ninvt_t = vslice(NT)
            invpc_t = vslice(NT)
            coefv_t = vslice(NT)
            asch_t = vslice(NT)
            bsch_t = vslice(NT)
            oneh_t = vslice(NT * C)
            incl_t = vslice(NT * NSLOT)
            eye_t = vslice(128)

            emb_t = bigp.tile([D, B], in_dt, tag="embT")
            bank_t = bigp.tile([D, M], in_dt, tag="bankT")

            # DMA triggers: sync + scalar HWDGE; the scalar ones all fire
            # before the exp stream starts.
            Q = B // 4
            H4 = M // 4
            nc.sync.dma_start(out=vecs_t[:], in_=vecs_d[:])
            nc.sync.dma_start(out=anch_t[:], in_=anchT_d[:])
            nc.sync.dma_start(out=emb_t[:, 0:Q], in_=embT_d[:, 0:Q])
            nc.sync.dma_start(out=emb_t[:, Q:2 * Q], in_=embT_d[:, Q:2 * Q])
            nc.scalar.dma_start(out=emb_t[:, 2 * Q:3 * Q], in_=embT_d[:, 2 * Q:3 * Q])
            nc.scalar.dma_start(out=emb_t[:, 3 * Q:B], in_=embT_d[:, 3 * Q:B])
            nc.sync.dma_start(out=bank_t[:, 0:H4], in_=bankT_d[:, 0:H4])
            nc.scalar.dma_start(out=bank_t[:, H4:2 * H4], in_=bankT_d[:, H4:2 * H4])
            nc.sync.dma_start(out=bank_t[:, 2 * H4:3 * H4], in_=bankT_d[:, 2 * H4:3 * H4])
            nc.scalar.dma_start(out=bank_t[:, 3 * H4:M], in_=bankT_d[:, 3 * H4:M])

            oout_t = smp.tile([128, 2 * NT], F32, tag="oout")
            scrA = smp.tile([128, G], BF16, tag="scrA")
            scrI = smp.tile([128, G], I32, tag="scrI")
            scrO2 = smp.tile([128, G], BF16, tag="scrO2")
            sdiag = [smp.tile([128, 1], F32, tag=f"sdiag{t}", name=f"sdiag{t}") for t in range(NT)]
            selfe = [smp.tile([128, 1], F32, tag=f"selfe{t}", name=f"selfe{t}") for t in range(NT)]
            eyemul = smp.tile([128, 128], F32, tag="eyemul")
            warm = smp.tile([128, 1], F32, tag="warm")
            raw3 = [smp.tile([128, C], F32, tag=f"raw3{t}", name=f"raw3{t}") for t in range(NT)]
            esum = [smp.tile([128, NSLOT], F32, tag=f"esum{t}", name=f"esum{t}") for t in range(NT)]
            scrNK = [smp.tile([128, NSLOT], F32, tag=f"scrNK{t}", name=f"scrNK{t}") for t in range(NT)]
            scrC = [smp.tile([128, C], F32, tag=f"scrC{t}", name=f"scrC{t}") for t in range(NT)]

            # exp table load ASAP (no DMA dependency)
            nc.vector.memset(junkw_t[:], 0.0)
            nc.vector.memset(junkx_t[:], 0.0)
            nc.scalar.activation(warm[:], junkw_t[:, 0:1], AF.Exp)

            def anch(t):
                return anch_t[:, t * 128:(t + 1) * 128]

            junk_ps = junkp.tile([128, CH], F32, tag="junk", name="junk_ps")

            def emit_junk(n):
                for _ in range(n):
                    nc.tensor.matmul(junk_ps[:], junkw_t[:], junkx_t[:],
                                     start=True, stop=True)

            # PE warmup while DMAs land
            emit_junk(8)

            # prelude in the pong buffer: diag blocks + per-class raw sums
            pre_ps = pongp.tile([128, P], F32, tag="pong", name="pre_ps")
            for t in range(NT):
                nc.tensor.matmul(
                    pre_ps[:, t * 128:(t + 1) * 128], anch(t), anch(t),
                    start=True, stop=True,
                )
            for t in range(NT):
                nc.tensor.matmul(
                    pre_ps[:, 256 + t * C:256 + (t + 1) * C], anch(t),
                    anch_t[:, APC:APC + C], start=True, stop=True,
                )
            for t in range(NT):
                nc.vector.tensor_mul(eyemul[:], pre_ps[:, t * 128:(t + 1) * 128], eye_t[:])
                nc.vector.reduce_sum(sdiag[t][:], eyemul[:], axis=AX.X)
                nc.vector.tensor_copy(out=raw3[t][:], in_=pre_ps[:, 256 + t * C:256 + (t + 1) * C])
                nc.scalar.activation(
                    selfe[t][:], sdiag[t][:], AF.Exp,
                    bias=ninvt_t[:, t:t + 1], scale=invt_t[:, t:t + 1],
                )

            def epi_early(t):
                """olin = coefv*invt*(1 - pos); runs during the stream."""
                own_r = smp.tile([128, 1], F32, tag=f"ownr{t}", name=f"ownr{t}")
                pos = smp.tile([128, 1], F32, tag=f"pos{t}", name=f"pos{t}")
                w1 = smp.tile([128, 1], F32, tag=f"w1{t}", name=f"w1{t}")
                nc.vector.tensor_mul(scrC[t][:], raw3[t][:], oneh_t[:, t * C:(t + 1) * C])
                nc.vector.reduce_sum(own_r[:], scrC[t][:], axis=AX.X)
                nc.vector.scalar_tensor_tensor(
                    out=pos[:], in0=own_r[:], scalar=sdiag[t][:], in1=invpc_t[:, t:t + 1],
                    op0=ALU.subtract, op1=ALU.mult,
                )
                nc.vector.scalar_tensor_tensor(
                    out=w1[:], in0=pos[:], scalar=-1.0, in1=invt_t[:, t:t + 1],
                    op0=ALU.mult, op1=ALU.mult,
                )
                nc.vector.scalar_tensor_tensor(
                    out=oout_t[:, NT + t:NT + t + 1], in0=w1[:], scalar=invt_t[:, t:t + 1],
                    in1=coefv_t[:, t:t + 1], op0=ALU.add, op1=ALU.mult,
                )

            def epilogue(t):
                """den = sum_k esum_k*incl_k - selfe."""
                nc.vector.tensor_mul(scrNK[t][:], esum[t][:], incl_t[:, t * NSLOT:(t + 1) * NSLOT])
                nc.vector.reduce_sum(oout_t[:, t:t + 1], scrNK[t][:], axis=AX.X)
                nc.vector.tensor_sub(oout_t[:, t:t + 1], oout_t[:, t:t + 1], selfe[t][:])

            def emit_chunk(t, ci, junk=True):
                buf, s, sz = chunks[ci]
                pool, tg, bw = (pingp, "ping", G) if buf == "G" else (pongp, "pong", P)
                ps = pool.tile([128, bw], F32, tag=tg, name=f"ps_t{t}_c{ci}")
                off = 0 if s < 0 else s
                src = emb_t if s < 0 else bank_t
                nmm = (sz + CH - 1) // CH
                for q in range(nmm):
                    a, b = q * CH, min((q + 1) * CH, sz)
                    nc.tensor.matmul(
                        ps[:, a:b], anch(t), src[:, off + a:off + b],
                        start=True, stop=True,
                    )
                if ci in dve_set:
                    nc.vector.tensor_scalar(
                        out=scrI[:, 0:sz], in0=ps[:, 0:sz],
                        scalar1=asch_t[:, t:t + 1], scalar2=bsch_t[:, t:t + 1],
                        op0=ALU.mult, op1=ALU.add,
                    )
                    for (a, b) in segs[ci]:
                        k = slot_idx[(ci, a)]
                        nc.vector.tensor_scalar(
                            out=scrO2[:, a:b], in0=scrI[:, a:b].bitcast(F32),
                            scalar1=1.0, scalar2=0.0,
                            op0=ALU.mult, op1=ALU.add,
                            accum_out=esum[t][:, k:k + 1],
                        )

                else:
                    for (a, b) in segs[ci]:
                        k = slot_idx[(ci, a)]
                        nc.scalar.activation(
                            scrA[:, a:b], ps[:, a:b], AF.Exp,
                            bias=ninvt_t[:, t:t + 1], scale=invt_t[:, t:t + 1],
                            accum_out=esum[t][:, k:k + 1],
                        )


            NCH = len(chunks)
            emit_chunk(0, 0)
            emit_chunk(0, 1)
            for t in range(NT):
                epi_early(t)
            for ci in range(2, NCH):
                emit_chunk(0, ci)
            emit_chunk(1, 0)
            emit_chunk(1, 1)
            epilogue(0)
            for ci in range(2, NCH):
                emit_chunk(1, ci)
            epilogue(1)

            nc.sync.dma_start(out=oout_d[:], in_=oout_t[:])

            # dependency-free filler matmuls, emitted LAST: the tile
            # scheduler pops the lowest-priority ready instruction when the
            # PE idles, so these land exactly in the fill-wait gaps and
            # keep the HAM activity window busy (PE stays at 2.4 GHz).
            emit_junk(N_JUNK * NCH * NT)

    nc.compile()
    return nc, slots, NSLOT


def _per_core_cols(vec, core):
    """[B] host vector -> [128, NT] tile for one core (col t, partition p)."""
    sl = vec[core * APC:(core + 1) * APC]
    return np.ascontiguousarray(sl.reshape(NT, 128).T).astype(np.float32)


def kernel(embeddings, labels, bank_embs, bank_labels, class_temps):
    global LAST_EXEC_TIME_NS
    import ml_dtypes

    emb = np.asarray(embeddings, dtype=np.float32)
    bank = np.asarray(bank_embs, dtype=np.float32)
    lab = np.asarray(labels).astype(np.int64).ravel()
    blab = np.asarray(bank_labels).astype(np.int64).ravel()
    ct = np.asarray(class_temps, dtype=np.float32).ravel()

    bord = np.argsort(lab, kind="stable")
    slab = lab[bord]
    mord = np.argsort(blab, kind="stable")
    cnt = np.bincount(lab, minlength=C)
    mcnt = np.bincount(blab, minlength=C)
    mk_b1, mk_b2 = int(mcnt[0]), int(mcnt[0] + mcnt[1])

    embT = np.ascontiguousarray(emb[bord].T)      # [D, B]
    bankT = np.ascontiguousarray(bank[mord].T)    # [D, M]
    if MM_MODE == "bf16":
        embT = embT.astype(ml_dtypes.bfloat16)
        bankT = bankT.astype(ml_dtypes.bfloat16)

    temps = ct[slab]
    inv_t = (1.0 / temps).astype(np.float32)
    use_dve = N_DVE > 0 and float(inv_t.max()) <= 40.0
    pos_cnt = cnt[slab] - 1
    invpc = (1.0 / np.maximum(pos_cnt, 1)).astype(np.float32)
    validf = (pos_cnt > 0).astype(np.float32)
    coefv = (BASE_TEMP / temps).astype(np.float32) * validf
    oneh = np.eye(C, dtype=np.float32)[slab]      # [B, 3]
    n_valid = int((pos_cnt > 0).sum())

    nc, slots, NSLOT = _build(mk_b1, mk_b2, N_DVE, use_dve)

    slot_cls = np.array([cls for (_, _, _, cls) in slots])
    incl_full = ((slot_cls[None, :] < 0) | (slot_cls[None, :] != slab[:, None])).astype(np.float32)
    eye128 = np.eye(128, dtype=np.float32)

    asch = (K_SCH * inv_t).astype(np.float32)
    bsch = (MAGIC - C_SCH - K_SCH * inv_t.astype(np.float64)).astype(np.float32)

    gT = np.stack([emb[bord][slab == c].sum(axis=0) for c in range(C)], axis=1)
    gT = np.ascontiguousarray(gT).astype(embT.dtype)

    in_maps = []
    for core in range(NCORES):
        asl = slice(core * APC, (core + 1) * APC)
        oh = oneh[asl].reshape(NT, 128, C).transpose(1, 0, 2).reshape(128, NT * C)
        ic = incl_full[asl].reshape(NT, 128, NSLOT).transpose(1, 0, 2).reshape(128, NT * NSLOT)
        vecs = np.concatenate([
            _per_core_cols(inv_t, core),
            _per_core_cols(-inv_t, core),
            _per_core_cols(invpc, core),
            _per_core_cols(coefv, core),
            _per_core_cols(asch, core),
            _per_core_cols(bsch, core),
            oh.astype(np.float32),
            ic.astype(np.float32),
            eye128,
        ], axis=1)
        in_maps.append({
            "embT": embT,
            "anchT": np.ascontiguousarray(np.concatenate([embT[:, asl], gT], axis=1)),
            "bankT": bankT,
            "vecs": np.ascontiguousarray(vecs),
        })

    trace = os.environ.get("SUPCON_TRACE", "0") == "1"
    if trace:
        trace = _install_trace_shim()
    res = run_bass_kernel_spmd(nc, in_maps, core_ids=list(range(NCORES)), trace=trace)
    LAST_EXEC_TIME_NS = res.exec_time_ns

    loss_sum = np.float64(0.0)
    for core in range(NCORES):
        oo = np.asarray(res.results[core]["oout"], dtype=np.float64)    # [128, 2*NT]
        den, lin = oo[:, :NT], oo[:, NT:]
        cf = _per_core_cols(coefv, core).astype(np.float64)
        loss_sum += (cf * np.log(den) + lin).sum()
    return np.float32(loss_sum / max(n_valid, 1))
